# revision 13
# baseline (speedup 1.0000x reference)
"""Joint Maximum Mean Discrepancy loss on 8 Trainium2 NeuronCores.

Math: for streams (s0,t0) and (s1,t1), the reference builds per-stream
Gaussian kernels K_r = exp(-gamma_r * dist_r) over feats_r = [src; tgt]
(N=8192 rows), takes their elementwise product, and returns
mean(s2s + t2t - 2*s2t) over the B x B blocks.

Device decomposition:
  exponent E_ij = 2*g0*(X0_i . X0_j) + 2*g1*(X1_i . X1_j) - c_i - c_j,
  c_i = g0*|X0_i|^2 + g1*|X1_i|^2, gamma_r from the closed form
  sum(dist_r) = 2*N*sum(sq_r) - 2*||colsum(X_r)||^2. The joint kernel is
  exp(E); the loss is a signed/weighted sum of exp over the 136 unordered
  512-row chunk-pair blocks (symmetry halves the N x N work).

  PSUM accumulates P = SCALE*E from two matmuls per [128,512] m-tile:
    - fp8 e4m3 DoubleRow over the 256 stream-0 rows (2 K-rows/partition):
      rows sqrt(2*g0*SCALE)*X0, layout [128, 2, 512] with
      tile[p,s,x] = W0T[s*128+p, x]
    - bf16 over 66 rows: [sqrt(2*g1*SCALE)*X1 (64) ; ones ; -SCALE*c]
      (lhs variant) vs [... ; -SCALE*c ; ones] (rhs variant)
  ScalarE applies Exp with scale=1/SCALE into SBUF bf16 (the pace-setting
  engine: 18 x ~1.97us); VectorE folds halves twice with tensor_add
  (2x bf16 mode) then tensor_reduce's [128,512] to a per-partition sum
  column (acc [128,18], ~1.8us/block - just under ScalarE).
  fp8 end-to-end loss error vs float64 is ~2e-3 (measured host-sim),
  well inside the 2e-2 budget.

Block cover (SPMD): a fixed 18-block pattern over 8 chunk "slots";
core k maps slot v to chunk (S[v] + 2k) mod 16, S = (0,1,2,3,4,5,8,9).
The 8 shifted copies tile all 120 chunk pairs + 16 loops: difference
classes d=1..7 x base-parity are hit exactly once (host weight 2),
d=8 pairs twice (weight 1), loops once (weight 1). The host applies
weight * sign (sign -1 iff exactly one chunk is a target chunk >= 8)
and reduces in float64.
"""

import os

import numpy as np

import concourse.bacc as bacc
import concourse.bass as bass
import concourse.mybir as mybir
import concourse.tile as tile
from concourse.bass_utils import run_bass_kernel_spmd

B = 4096
D0, D1 = 256, 64
N = 2 * B
CH = 512          # rows per chunk
NCHUNK = 16
NCORE = 8
MT = 128          # m-tile rows
NMT = CH // MT    # m-tiles per block row (4)
SCALE = 64.0      # exponent pre-scale; exp applies 1/SCALE
KB = D1 + 2       # extra contraction rows (66)
GP = 33           # partitions of the 66-row DR tile

# cyclic support: slot v of core k is chunk (S[v] + 2k) % 16
S_SUPPORT = (0, 1, 2, 3, 4, 5, 8, 9)
NSLOT = 8
# 18-block pattern in slot indices, ordered so early blocks touch early
# slots (DMA pipelining): loops (0,0),(1,1); one pair per (diff 1..7,
# parity) class; both d=8 classes.
PATTERN = [
    (0, 0), (1, 1), (0, 1),
    (1, 2), (0, 2),
    (1, 3), (0, 3),
    (1, 4), (0, 4),
    (1, 5), (0, 5),
    (3, 6), (2, 6), (1, 6), (0, 6),
    (3, 7), (2, 7), (1, 7),
]
NBLK = len(PATTERN)  # 18

F8 = mybir.dt.float8e4
BF = mybir.dt.bfloat16
F32 = mybir.dt.float32

_N_WARMUP = int(os.environ.get("JMMD_WARMUP", "30"))

LAST_EXEC_NS = None
LAST_RESULTS = None

_CACHE: dict = {}


def _build():
    if "nc" in _CACHE:
        return _CACHE["nc"]
    nc = bacc.Bacc(
        "TRN2", target_bir_lowering=False, debug=False, enable_asserts=False
    )
    f8_dram = nc.dram_tensor("f8", [NSLOT, MT, 2, CH], F8, kind="ExternalInput").ap()
    g_dram = nc.dram_tensor("g", [NSLOT, GP, 2, 2 * CH], F8, kind="ExternalInput").ap()
    acc_dram = nc.dram_tensor("acc", [MT, NBLK + 1], F32, kind="ExternalOutput").ap()

    with tile.TileContext(nc) as tc:
        with (
            tc.tile_pool(name="const", bufs=1) as const,
            tc.tile_pool(name="exp", bufs=2) as expp,
            tc.tile_pool(name="red", bufs=2) as redp,
            tc.tile_pool(name="psum", bufs=2, space=bass.MemorySpace.PSUM) as psum,
        ):
            # warmup sources via gpsimd memset, queued BEFORE its DMAs —
            # memsets run in the pre-BSP window so the HAM warmup and ACT
            # table preload start as early as the engines are up.
            wz = const.tile([MT, 8], BF, tag="wz")
            w8 = const.tile([MT, 2, MT], F8, tag="w8")
            nc.gpsimd.memset(w8[:], 0.0)
            nc.gpsimd.memset(wz[:], 0.0)

            ft, gt = {}, {}
            for j in range(NSLOT):
                ft[j] = const.tile([MT, 2, CH], F8, name=f"f{j}", tag=f"f{j}")
                gt[j] = const.tile([GP, 2, 2 * CH], F8, name=f"g{j}", tag=f"g{j}")
                if j == 0:
                    qa, qb = nc.scalar, nc.scalar
                else:
                    qa, qb = (nc.sync, nc.gpsimd) if j % 2 == 0 else (nc.gpsimd, nc.sync)
                qa.dma_start(ft[j][:], f8_dram[j])
                qb.dma_start(gt[j][:], g_dram[j])

            acc_t = const.tile([MT, NBLK + 1], F32, tag="acc")

            # Exp ACT-table preload while input DMAs stream
            warm_act = const.tile([MT, 8], BF, tag="warm_act")
            nc.scalar.activation(
                warm_act[:], wz[:], mybir.ActivationFunctionType.Exp
            )

            # HAM warmup: dummy fp8-DR matmuls spanning > the 3.4us HAM
            # activity window so real matmuls start at the warm PE clock.
            if _N_WARMUP:
                warm_ps = psum.tile([MT, NMT * CH], F32, tag="ps")
                for _ in range(_N_WARMUP):
                    nc.tensor.matmul(
                        warm_ps[:, :MT],
                        w8[:],
                        w8[:],
                        start=True,
                        stop=True,
                        perf_mode=mybir.MatmulPerfMode.DoubleRow,
                    )

            HF = NMT * CH // 2
            for col, (r, c) in enumerate(PATTERN):
                ps = psum.tile([MT, NMT * CH], F32, tag="ps")
                for m in range(NMT):
                    nc.tensor.matmul(
                        ps[:, m * CH:(m + 1) * CH],
                        ft[r][:, :, m * MT:(m + 1) * MT],
                        ft[c][:],
                        start=True,
                        stop=False,
                        perf_mode=mybir.MatmulPerfMode.DoubleRow,
                    )
                for m in range(NMT):
                    nc.tensor.matmul(
                        ps[:, m * CH:(m + 1) * CH],
                        gt[r][:, :, m * MT:(m + 1) * MT],
                        gt[c][:, :, CH:],
                        start=False,
                        stop=True,
                        perf_mode=mybir.MatmulPerfMode.DoubleRow,
                    )
                if col == 0:
                    # chain starter: two half activations so ScalarE begins
                    # after m-tiles 0-1 instead of the whole block
                    ex = expp.tile([MT, NMT * CH], BF, tag="ex")
                    for h, ac in ((0, 0), (1, NBLK)):
                        nc.scalar.activation(
                            ex[:, h * HF:(h + 1) * HF],
                            ps[:, h * HF:(h + 1) * HF],
                            mybir.ActivationFunctionType.Exp,
                            scale=1.0 / SCALE,
                        )
                        red = redp.tile([MT, HF // 2], BF, tag="red")
                        nc.vector.tensor_add(
                            red[:],
                            ex[:, h * HF:h * HF + HF // 2],
                            ex[:, h * HF + HF // 2:(h + 1) * HF],
                        )
                        nc.vector.tensor_reduce(
                            acc_t[:, ac:ac + 1],
                            red[:],
                            axis=mybir.AxisListType.X,
                            op=mybir.AluOpType.add,
                        )
                elif col == NBLK - 1:
                    # chain finisher: accum_out on ScalarE, no vector tail
                    ex = expp.tile([MT, NMT * CH], BF, tag="ex")
                    nc.scalar.activation(
                        ex[:],
                        ps[:],
                        mybir.ActivationFunctionType.Exp,
                        scale=1.0 / SCALE,
                        accum_out=acc_t[:, col:col + 1],
                    )
                else:
                    ex = expp.tile([MT, NMT * CH], BF, tag="ex")
                    nc.scalar.activation(
                        ex[:], ps[:], mybir.ActivationFunctionType.Exp,
                        scale=1.0 / SCALE,
                    )
                    red = redp.tile([MT, HF], BF, tag="red")
                    nc.vector.tensor_add(red[:], ex[:, :HF], ex[:, HF:])
                    red2 = redp.tile([MT, HF // 2], BF, tag="red2")
                    nc.vector.tensor_add(
                        red2[:], red[:, :HF // 2], red[:, HF // 2:]
                    )
                    nc.vector.tensor_reduce(
                        acc_t[:, col:col + 1],
                        red2[:],
                        axis=mybir.AxisListType.X,
                        op=mybir.AluOpType.add,
                    )
            nc.sync.dma_start(acc_dram, acc_t[:])
    nc.compile()
    _CACHE["nc"] = nc
    return nc


def _dr_pack(Wrows):
    """[2*P, X] contraction rows -> DR tile [P, 2, X] with
    tile[p, s, x] = Wrows[s*P + p, x]."""
    P = Wrows.shape[0] // 2
    return np.ascontiguousarray(
        Wrows.reshape(2, P, Wrows.shape[1]).transpose(1, 0, 2)
    )


def _pack_inputs(s0, s1, t0, t1):
    import ml_dtypes

    X0 = np.concatenate([s0, t0], axis=0).astype(np.float64)
    X1 = np.concatenate([s1, t1], axis=0).astype(np.float64)

    def gamma_of(X):
        sq = np.sum(X * X, axis=1)
        sdist = 2.0 * N * np.sum(sq) - 2.0 * np.sum(np.sum(X, axis=0) ** 2)
        return (N * N - N) / sdist, sq

    g0, sq0 = gamma_of(X0)
    g1, sq1 = gamma_of(X1)
    c = g0 * sq0 + g1 * sq1

    f8 = ml_dtypes.float8_e4m3
    W0 = np.clip(np.sqrt(2.0 * g0 * SCALE) * X0, -240, 240).astype(f8)
    W1 = np.clip(np.sqrt(2.0 * g1 * SCALE) * X1, -240, 240).astype(f8)
    cq = np.clip(-SCALE * c, -240, 240).astype(f8)

    fch, gch = [], []
    for ch in range(NCHUNK):
        rows = slice(ch * CH, (ch + 1) * CH)
        fch.append(_dr_pack(W0[rows].T))           # [128, 2, 512]
        g = np.empty((KB, 2 * CH), dtype=f8)
        g[:D1, :CH] = W1[rows].T
        g[:D1, CH:] = W1[rows].T
        g[D1, :CH] = 1.0
        g[D1 + 1, :CH] = cq[rows]
        g[D1, CH:] = cq[rows]
        g[D1 + 1, CH:] = 1.0
        gch.append(np.ascontiguousarray(
            g.reshape(2, GP, 2 * CH).transpose(1, 0, 2)))

    in_maps = []
    for k in range(NCORE):
        slots = [(S_SUPPORT[v] + 2 * k) % NCHUNK for v in range(NSLOT)]
        in_maps.append(
            {
                "f8": np.ascontiguousarray(np.stack([fch[ch] for ch in slots])),
                "g": np.ascontiguousarray(np.stack([gch[ch] for ch in slots])),
            }
        )
    return in_maps


def _combine(results):
    total = 0.0
    for k in range(NCORE):
        acc = np.asarray(results[k]["acc"], dtype=np.float64)  # [128, NBLK+1]
        colsum = acc.sum(axis=0)
        colsum[0] += colsum[NBLK]
        for col, (r, c) in enumerate(PATTERN):
            u = (S_SUPPORT[r] + 2 * k) % NCHUNK
            v = (S_SUPPORT[c] + 2 * k) % NCHUNK
            d = min((v - u) % NCHUNK, (u - v) % NCHUNK)
            w = 2.0 if 0 < d < 8 else 1.0        # loops and d=8 (doubled): 1
            s = (1.0 if u < 8 else -1.0) * (1.0 if v < 8 else -1.0)
            total += w * s * colsum[col]
    return total / (B * B)


def kernel(s0, s1, t0, t1):
    global LAST_EXEC_NS, LAST_RESULTS
    nc = _build()
    in_maps = _pack_inputs(
        np.asarray(s0), np.asarray(s1), np.asarray(t0), np.asarray(t1)
    )
    trace = os.environ.get("JMMD_TRACE", "0") == "1"
    res = run_bass_kernel_spmd(nc, in_maps, core_ids=list(range(NCORE)), trace=trace)
    LAST_EXEC_NS = res.exec_time_ns
    LAST_RESULTS = res
    return np.float32(_combine(res.results))


# revision 14
# speedup vs baseline: 1.8136x; 1.8136x over previous
"""Joint Maximum Mean Discrepancy loss on 8 Trainium2 NeuronCores.

Math: for streams (s0,t0) and (s1,t1), the reference builds per-stream
Gaussian kernels K_r = exp(-gamma_r * dist_r) over feats_r = [src; tgt]
(N=8192 rows), takes their elementwise product, and returns
mean(s2s + t2t - 2*s2t) over the B x B blocks.

Device decomposition:
  exponent E_ij = 2*g0*(X0_i . X0_j) + 2*g1*(X1_i . X1_j) - c_i - c_j,
  c_i = g0*|X0_i|^2 + g1*|X1_i|^2, gamma_r from the closed form
  sum(dist_r) = 2*N*sum(sq_r) - 2*||colsum(X_r)||^2. The joint kernel is
  exp(E); the loss is a signed/weighted sum of exp over the 136 unordered
  512-row chunk-pair blocks (symmetry halves the N x N work).

  PSUM accumulates P = SCALE*E from two matmuls per [128,512] m-tile:
    - fp8 e4m3 DoubleRow over the 256 stream-0 rows (2 K-rows/partition):
      rows sqrt(2*g0*SCALE)*X0, layout [128, 2, 512] with
      tile[p,s,x] = W0T[s*128+p, x]
    - bf16 over 66 rows: [sqrt(2*g1*SCALE)*X1 (64) ; ones ; -SCALE*c]
      (lhs variant) vs [... ; -SCALE*c ; ones] (rhs variant)
  ScalarE applies Exp with scale=1/SCALE into SBUF bf16 (the pace-setting
  engine: 18 x ~1.97us); VectorE folds halves twice with tensor_add
  (2x bf16 mode) then tensor_reduce's [128,512] to a per-partition sum
  column (acc [128,18], ~1.8us/block - just under ScalarE).
  fp8 end-to-end loss error vs float64 is ~2e-3 (measured host-sim),
  well inside the 2e-2 budget.

Block cover (SPMD): a fixed 18-block pattern over 8 chunk "slots";
core k maps slot v to chunk (S[v] + 2k) mod 16, S = (0,1,2,3,4,5,8,9).
The 8 shifted copies tile all 120 chunk pairs + 16 loops: difference
classes d=1..7 x base-parity are hit exactly once (host weight 2),
d=8 pairs twice (weight 1), loops once (weight 1). The host applies
weight * sign (sign -1 iff exactly one chunk is a target chunk >= 8)
and reduces in float64.
"""

import os

import numpy as np

import concourse.bacc as bacc
import concourse.bass as bass
import concourse.mybir as mybir
import concourse.tile as tile
from concourse.bass_utils import run_bass_kernel_spmd

B = 4096
D0, D1 = 256, 64
N = 2 * B
CH = 512          # rows per chunk
NCHUNK = 16
NCORE = 8
MT = 128          # m-tile rows
NMT = CH // MT    # m-tiles per block row (4)
SCALE = 64.0      # exponent pre-scale; exp applies 1/SCALE
KB = D1 + 2       # bf16 contraction rows (66)

# cyclic support: slot v of core k is chunk (S[v] + 2k) % 16
S_SUPPORT = (0, 1, 2, 3, 4, 5, 8, 9)
NSLOT = 8
# 18-block pattern in slot indices, ordered so early blocks touch early
# slots (DMA pipelining): loops (0,0),(1,1); one pair per (diff 1..7,
# parity) class; both d=8 classes.
PATTERN = [
    (0, 0), (1, 1), (0, 1),
    (1, 2), (0, 2),
    (1, 3), (0, 3),
    (1, 4), (0, 4),
    (1, 5), (0, 5),
    (3, 6), (2, 6), (1, 6), (0, 6),
    (3, 7), (2, 7), (1, 7),
]
NBLK = len(PATTERN)  # 18

F8 = mybir.dt.float8e4
BF = mybir.dt.bfloat16
F32 = mybir.dt.float32

_N_WARMUP = int(os.environ.get("JMMD_WARMUP", "30"))

LAST_EXEC_NS = None
LAST_RESULTS = None

_CACHE: dict = {}


def _build():
    if "nc" in _CACHE:
        return _CACHE["nc"]
    nc = bacc.Bacc(
        "TRN2", target_bir_lowering=False, debug=False, enable_asserts=False
    )
    f8_dram = nc.dram_tensor("f8", [NSLOT, MT, 2, CH], F8, kind="ExternalInput").ap()
    g_dram = nc.dram_tensor("g", [NSLOT, KB, 2 * CH], BF, kind="ExternalInput").ap()
    acc_dram = nc.dram_tensor("acc", [MT, NBLK + 1], F32, kind="ExternalOutput").ap()

    with tile.TileContext(nc) as tc:
        with (
            tc.tile_pool(name="const", bufs=1) as const,
            tc.tile_pool(name="exp", bufs=2) as expp,
            tc.tile_pool(name="red", bufs=2) as redp,
            tc.tile_pool(name="psum", bufs=2, space=bass.MemorySpace.PSUM) as psum,
        ):
            # warmup sources via gpsimd memset, queued BEFORE its DMAs —
            # memsets run in the pre-BSP window so the HAM warmup and ACT
            # table preload start as early as the engines are up.
            wz = const.tile([MT, 8], BF, tag="wz")
            w8 = const.tile([MT, 2, MT], F8, tag="w8")
            nc.gpsimd.memset(w8[:], 0.0)
            nc.gpsimd.memset(wz[:], 0.0)

            ft, gt = {}, {}
            for j in range(NSLOT):
                ft[j] = const.tile([MT, 2, CH], F8, name=f"f{j}", tag=f"f{j}")
                gt[j] = const.tile([KB, 2 * CH], BF, name=f"g{j}", tag=f"g{j}")
                if j == 0:
                    qa, qb = nc.scalar, nc.scalar
                else:
                    qa, qb = (nc.sync, nc.gpsimd) if j % 2 == 0 else (nc.gpsimd, nc.sync)
                qa.dma_start(ft[j][:], f8_dram[j])
                qb.dma_start(gt[j][:], g_dram[j])

            acc_t = const.tile([MT, NBLK + 1], F32, tag="acc")

            # Exp ACT-table preload while input DMAs stream
            warm_act = const.tile([MT, 8], BF, tag="warm_act")
            nc.scalar.activation(
                warm_act[:], wz[:], mybir.ActivationFunctionType.Exp
            )

            # HAM warmup: dummy fp8-DR matmuls spanning > the 3.4us HAM
            # activity window so real matmuls start at the warm PE clock.
            if _N_WARMUP:
                warm_ps = psum.tile([MT, NMT * CH], F32, tag="ps")
                for _ in range(_N_WARMUP):
                    nc.tensor.matmul(
                        warm_ps[:, :MT],
                        w8[:],
                        w8[:],
                        start=True,
                        stop=True,
                        perf_mode=mybir.MatmulPerfMode.DoubleRow,
                    )

            HF = NMT * CH // 2
            for col, (r, c) in enumerate(PATTERN):
                ps = psum.tile([MT, NMT * CH], F32, tag="ps")
                for m in range(NMT):
                    nc.tensor.matmul(
                        ps[:, m * CH:(m + 1) * CH],
                        ft[r][:, :, m * MT:(m + 1) * MT],
                        ft[c][:],
                        start=True,
                        stop=False,
                        perf_mode=mybir.MatmulPerfMode.DoubleRow,
                    )
                for m in range(NMT):
                    nc.tensor.matmul(
                        ps[:, m * CH:(m + 1) * CH],
                        gt[r][:, m * MT:(m + 1) * MT],
                        gt[c][:, CH:],
                        start=False,
                        stop=True,
                    )
                if col == 0:
                    # chain starter: two half activations so ScalarE begins
                    # after m-tiles 0-1 instead of the whole block
                    ex = expp.tile([MT, NMT * CH], BF, tag="ex")
                    for h, ac in ((0, 0), (1, NBLK)):
                        nc.scalar.activation(
                            ex[:, h * HF:(h + 1) * HF],
                            ps[:, h * HF:(h + 1) * HF],
                            mybir.ActivationFunctionType.Exp,
                            scale=1.0 / SCALE,
                        )
                        red = redp.tile([MT, HF // 2], BF, tag="red")
                        nc.vector.tensor_add(
                            red[:],
                            ex[:, h * HF:h * HF + HF // 2],
                            ex[:, h * HF + HF // 2:(h + 1) * HF],
                        )
                        nc.vector.tensor_reduce(
                            acc_t[:, ac:ac + 1],
                            red[:],
                            axis=mybir.AxisListType.X,
                            op=mybir.AluOpType.add,
                        )
                elif col == NBLK - 1:
                    # chain finisher: accum_out on ScalarE, no vector tail
                    ex = expp.tile([MT, NMT * CH], BF, tag="ex")
                    nc.scalar.activation(
                        ex[:],
                        ps[:],
                        mybir.ActivationFunctionType.Exp,
                        scale=1.0 / SCALE,
                        accum_out=acc_t[:, col:col + 1],
                    )
                else:
                    ex = expp.tile([MT, NMT * CH], BF, tag="ex")
                    nc.scalar.activation(
                        ex[:], ps[:], mybir.ActivationFunctionType.Exp,
                        scale=1.0 / SCALE,
                    )
                    red = redp.tile([MT, HF], BF, tag="red")
                    nc.vector.tensor_add(red[:], ex[:, :HF], ex[:, HF:])
                    red2 = redp.tile([MT, HF // 2], BF, tag="red2")
                    nc.vector.tensor_add(
                        red2[:], red[:, :HF // 2], red[:, HF // 2:]
                    )
                    nc.vector.tensor_reduce(
                        acc_t[:, col:col + 1],
                        red2[:],
                        axis=mybir.AxisListType.X,
                        op=mybir.AluOpType.add,
                    )
            nc.sync.dma_start(acc_dram, acc_t[:])
    nc.compile()
    _CACHE["nc"] = nc
    return nc


def _dr_pack(Wrows):
    """[2*P, X] contraction rows -> DR tile [P, 2, X] with
    tile[p, s, x] = Wrows[s*P + p, x]."""
    P = Wrows.shape[0] // 2
    return np.ascontiguousarray(
        Wrows.reshape(2, P, Wrows.shape[1]).transpose(1, 0, 2)
    )


def _pack_inputs(s0, s1, t0, t1):
    import ml_dtypes

    X0 = np.concatenate([s0, t0], axis=0).astype(np.float64)
    X1 = np.concatenate([s1, t1], axis=0).astype(np.float64)

    def gamma_of(X):
        sq = np.sum(X * X, axis=1)
        sdist = 2.0 * N * np.sum(sq) - 2.0 * np.sum(np.sum(X, axis=0) ** 2)
        return (N * N - N) / sdist, sq

    g0, sq0 = gamma_of(X0)
    g1, sq1 = gamma_of(X1)
    c = g0 * sq0 + g1 * sq1

    f8 = ml_dtypes.float8_e4m3
    W0 = np.clip(np.sqrt(2.0 * g0 * SCALE) * X0, -240, 240).astype(f8)
    W1 = (np.sqrt(2.0 * g1 * SCALE) * X1).astype(ml_dtypes.bfloat16)
    cq = (-SCALE * c).astype(ml_dtypes.bfloat16)

    fch, gch = [], []
    for ch in range(NCHUNK):
        rows = slice(ch * CH, (ch + 1) * CH)
        fch.append(_dr_pack(W0[rows].T))           # [128, 2, 512]
        g = np.empty((KB, 2 * CH), dtype=ml_dtypes.bfloat16)
        g[:D1, :CH] = W1[rows].T
        g[:D1, CH:] = W1[rows].T
        g[D1, :CH] = 1.0
        g[D1 + 1, :CH] = cq[rows]
        g[D1, CH:] = cq[rows]
        g[D1 + 1, CH:] = 1.0
        gch.append(g)

    in_maps = []
    for k in range(NCORE):
        slots = [(S_SUPPORT[v] + 2 * k) % NCHUNK for v in range(NSLOT)]
        in_maps.append(
            {
                "f8": np.ascontiguousarray(np.stack([fch[ch] for ch in slots])),
                "g": np.ascontiguousarray(np.stack([gch[ch] for ch in slots])),
            }
        )
    return in_maps


def _combine(results):
    total = 0.0
    for k in range(NCORE):
        acc = np.asarray(results[k]["acc"], dtype=np.float64)  # [128, NBLK+1]
        colsum = acc.sum(axis=0)
        colsum[0] += colsum[NBLK]
        for col, (r, c) in enumerate(PATTERN):
            u = (S_SUPPORT[r] + 2 * k) % NCHUNK
            v = (S_SUPPORT[c] + 2 * k) % NCHUNK
            d = min((v - u) % NCHUNK, (u - v) % NCHUNK)
            w = 2.0 if 0 < d < 8 else 1.0        # loops and d=8 (doubled): 1
            s = (1.0 if u < 8 else -1.0) * (1.0 if v < 8 else -1.0)
            total += w * s * colsum[col]
    return total / (B * B)


def kernel(s0, s1, t0, t1):
    global LAST_EXEC_NS, LAST_RESULTS
    nc = _build()
    in_maps = _pack_inputs(
        np.asarray(s0), np.asarray(s1), np.asarray(t0), np.asarray(t1)
    )
    trace = os.environ.get("JMMD_TRACE", "0") == "1"
    res = run_bass_kernel_spmd(nc, in_maps, core_ids=list(range(NCORE)), trace=trace)
    LAST_EXEC_NS = res.exec_time_ns
    LAST_RESULTS = res
    return np.float32(_combine(res.results))
